# revision 42
# baseline (speedup 1.0000x reference)
"""DeepseekV2 MLA attention (B=1, S=2048, H=4096, NH=32) on 8 TRN2 cores.

v2: bf16 end-to-end (tolerance 2e-2), restructured for PE efficiency.

Sharding:
 - Front (q_a + kv_a + k_pe rope): data-parallel over sequence.  Each core
   projects its 256-column slice of hidden through the FULL front weight
   (17 M-tiles), RMS-normalizes locally, applies k rope locally, then ONE
   AllGather shares the normalized latent [2176, 256] (bf16) with all cores.
 - Back (Wqb / Wkvb / attention): tensor-parallel over heads (4 heads/core),
   consuming the gathered latent.
 - Wo: contraction-sharded (each core's 4-head slice of the Wo contraction);
   bf16 partial [4096, 2048] written per core, host sums the 8 partials.

Causal masking is done with affine_select (iota compare) on the exp tiles of
the 4 diagonal blocks per query chunk -- zero mask DMA.  A generic fallback
path (mask as data) is built when the host detects a non-causal mask.

All matmuls are bf16 (full-rate PE + fast weight load); accumulation fp32 in
PSUM.  Softmax denominators accumulate in fp32 on DVE.
"""

import os
import numpy as np
import ml_dtypes

import concourse.bass as bass
import concourse.mybir as mybir
from concourse.tile import TileContext
import concourse.bass_utils as bass_utils
from concourse.bass_utils import run_bass_kernel_spmd

bass_utils.upload_artifacts = lambda tmpdir: tmpdir  # no artifact bucket here

S = 2048
H = 4096
NCORES = 8
NHC = 4            # heads per core
NOPE, ROPE, VD = 128, 64, 128
QHD = NOPE + ROPE  # 192
QLR, KVLR = 1536, 512
BASE = 10000.0
EPS = 1e-6
SCALE = QHD ** -0.5
P = 128
SC = 512           # seq chunk
SLC = S // NCORES  # 256 per-core front slice
NSC = S // SC      # 4
NKB = S // P       # 16 key blocks
BF = mybir.dt.bfloat16
FR = mybir.dt.float32r
F32 = mybir.dt.float32
AF = mybir.ActivationFunctionType
NPBF = ml_dtypes.bfloat16

N_KI = H // P      # 32 front contraction tiles
NQB = QLR // P     # 12
NKVB = KVLR // P   # 4
N_FB = NQB + NKVB + 1   # 17 front output blocks (12 q + 4 kv + 1 rope(64))
CCR = N_FB * P     # 2176 gathered rows (incl 64 pad)


def split_multiwaits(nc, cap=1):
    """Walrus pin: only `cap` sync-waits per instruction; spill extras onto
    same-engine NoOps inserted just before the instruction."""
    for f in nc.m.functions:
        for b in f.blocks:
            li = b.instructions
            out = []
            changed = False
            for inst in list(li):
                si = getattr(inst, "sync_info", None)
                waits = list(si.on_wait) if si is not None and si.on_wait else []
                if len(waits) > cap:
                    changed = True
                    extra, keep = waits[:-cap], waits[-cap:]
                    for j in range(0, len(extra), cap):
                        out.append(mybir.InstNoOp(
                            name=nc.get_next_instruction_name(),
                            engine=inst.engine, ins=[], outs=[],
                            sync_info=mybir.SyncInfo(
                                on_wait=extra[j:j + cap], on_update=[]),
                            bass_nofuse=True,
                        ))
                    inst.sync_info = mybir.SyncInfo(
                        on_wait=keep, on_update=list(si.on_update))
                out.append(inst)
            if changed:
                li[:] = out


def build(causal: bool) -> bass.Bass:
    nc = bass.Bass()
    hTs = nc.declare_dram_parameter("hTs", [H, SLC], BF, isOutput=False)
    Wp = nc.declare_dram_parameter("Wp", [P, N_FB * N_KI * P], BF, isOutput=False)
    Wqb_p = nc.declare_dram_parameter("Wqb_p", [P, NQB * NHC * QHD], BF, isOutput=False)
    Wkvb_p = nc.declare_dram_parameter("Wkvb_p", [P, NKVB * NHC * (NOPE + VD)], BF, isOutput=False)
    # Wkvb_p k-tile layout: [h0n h1n h2n h3n | h0v h1v h2v h3v] (nope then vd)
    Wo_p = nc.declare_dram_parameter("Wo_p", [P, NKVB * H], BF, isOutput=False)
    cqp = nc.declare_dram_parameter("cqp", [P, S], BF, isOutput=False)
    sqp = nc.declare_dram_parameter("sqp", [P, S], BF, isOutput=False)
    cql = nc.declare_dram_parameter("cql", [ROPE, SLC], BF, isOutput=False)
    sql = nc.declare_dram_parameter("sql", [ROPE, SLC], BF, isOutput=False)
    if not causal:
        maskT = nc.declare_dram_parameter("maskT", [S, S], BF, isOutput=False)
    outT = nc.declare_dram_parameter("outT", [P, (H // P) * S], BF, isOutput=True)

    Wp3 = Wp.rearrange("p (fk w) -> p fk w", w=P)          # [P, 17*32, 128]
    Wqb3 = Wqb_p.rearrange("p (k w) -> p k w", k=NQB)      # [P, 12, 768]
    Wkvb3 = Wkvb_p.rearrange("p (k w) -> p k w", k=NKVB)   # [P, 4, 1024]
    Wo3 = Wo_p.rearrange("p (k w) -> p k w", k=NKVB)       # [P, 4, 4096]
    out3 = outT.rearrange("p (ho s) -> p ho s", s=S)       # [P, 32, 2048]
    h3 = hTs.rearrange("(k p) c -> p k c", p=P)            # [P, 32, 256]

    with TileContext(nc) as tc:
        with (
            tc.tile_pool(name="dram", bufs=1, space="DRAM") as dpool,
            tc.tile_pool(name="const", bufs=1) as cpool,
        ):
            NKVF = NKVB + 1   # kv front blocks incl rope
            cc_kv = dpool.tile([P, NKVF * SLC], BF, tag="cckv")
            cc_kv_out = dpool.tile([NCORES, P, NKVF * SLC], BF,
                                   addr_space="Shared", tag="cckvo")
            cc_q = dpool.tile([P, NQB * SLC], BF, tag="ccq")
            cc_q_out = dpool.tile([NCORES, P, NQB * SLC], BF,
                                  addr_space="Shared", tag="ccqo")

            ones_bf = cpool.tile([P, 1], BF)
            nc.vector.memset(ones_bf[:], 1.0)
            ones_cf = cpool.tile([P, 1], F32)
            nc.vector.memset(ones_cf[:], 1.0)
            ones_col = cpool.tile([P, 1], FR)
            nc.scalar.copy(ones_col[:], ones_cf[:])
            ones_rf = cpool.tile([NHC, P], F32)
            nc.vector.memset(ones_rf[:], 1.0)
            ones_row4 = cpool.tile([NHC, P], FR)
            nc.scalar.copy(ones_row4[:], ones_rf[:])
            ones_row = cpool.tile([1, P], FR)
            nc.scalar.copy(ones_row[:], ones_rf[0:1, :])
            cqp_t = cpool.tile([P, S], BF, tag="cqp")
            sqp_t = cpool.tile([P, S], BF, tag="sqp")
            nc.sync.dma_start(out=cqp_t[:], in_=cqp[:, :])
            nc.sync.dma_start(out=sqp_t[:], in_=sqp[:, :])
            # causal staircase mask: tri(r, c) = 1 iff c >= r.  Every diagonal
            # block's mask is tri[:, :n] under the column restriction.
            tri = cpool.tile([P, SC], BF, tag="tri")
            nc.vector.memset(tri[:], 1.0)
            nc.gpsimd.affine_select(
                tri[:], tri[:], pattern=[[1, SC]],
                compare_op=mybir.AluOpType.is_ge,
                fill=0.0, base=0, channel_multiplier=-1)

            # ------------- Phase 1: front projections on local seq slice
            with (
                tc.tile_pool(name="hcol", bufs=1) as hpool,
                tc.tile_pool(name="wfr", bufs=3) as wpool,
                tc.tile_pool(name="raw", bufs=1) as rpool,
                tc.tile_pool(name="nrm", bufs=2) as npool,
                tc.tile_pool(name="stg", bufs=1) as spool,
                tc.tile_pool(name="ps", bufs=3, space="PSUM") as pspool,
                tc.tile_pool(name="ps1", bufs=1, space="PSUM") as ps1pool,
            ):
                h_sb = hpool.tile([P, N_KI, SLC], BF, tag="h")
                nc.sync.dma_start(out=h_sb[:], in_=h3[:, :, :])
                stg_kv = spool.tile([P, NKVF, SLC], BF, tag="stgkv")
                nc.vector.memset(stg_kv[ROPE:P, NKVB, :], 0.0)  # rope blk pad rows
                stg_q = spool.tile([P, NQB, SLC], BF, tag="stgq")

                # kv blocks first so the kv AllGather overlaps the q-front GEMMs
                raws = {}
                sq_q = ps1pool.tile([1, SLC], F32, tag="sqq")
                sq_kv = ps1pool.tile([1, SLC], F32, tag="sqkv")
                fb_order = list(range(NQB, N_FB)) + list(range(NQB))
                sqts = {}

                def front_block(fb):
                    w = ROPE if fb == N_FB - 1 else P
                    wt = wpool.tile([P, N_KI, P], BF, tag="w", name=f"w{fb}")
                    nc.sync.dma_start(
                        out=wt[:], in_=Wp3[:, fb * N_KI:(fb + 1) * N_KI, :])
                    ps = pspool.tile([P, SLC], F32, tag="ps", name=f"ps{fb}")
                    for ki in range(N_KI):
                        nc.tensor.matmul(ps[:w, :], lhsT=wt[:, ki, :w], rhs=h_sb[:, ki, :],
                                         start=(ki == 0), stop=(ki == N_KI - 1))
                    raw = rpool.tile([P, SLC], BF, tag=f"r{fb}", name=f"r{fb}")
                    nc.scalar.copy(raw[:w, :], ps[:w, :])
                    raws[fb] = raw
                    if fb < NQB + NKVB:
                        # square on DVE now; partition-sum matmuls are batched
                        # later so they don't stall the PE pipeline
                        sqt = rpool.tile([P, SLC], BF, tag=f"sq{fb}", name=f"sqt{fb}")
                        nc.vector.tensor_mul(sqt[:], raw[:], raw[:])
                        sqts[fb] = sqt

                def norm_chain(sq_ps, denom, name):
                    ms = npool.tile([1, SLC], F32, tag="ms", name=f"ms{name}")
                    nc.scalar.activation(ms[:], sq_ps[:], AF.Copy,
                                         scale=1.0 / denom, bias=EPS)
                    rc = npool.tile([1, SLC], F32, tag="rc", name=f"rc{name}")
                    nc.vector.reciprocal(rc[:], ms[:])
                    rs = npool.tile([1, SLC], FR, tag="rs", name=f"rs{name}")
                    nc.scalar.activation(rs[:], rc[:], AF.Sqrt)
                    return rs

                # kv-front blocks
                for fb in range(NQB, N_FB):
                    front_block(fb)
                for j in range(NKVB):
                    nc.tensor.matmul(sq_kv[:], lhsT=ones_bf[:], rhs=sqts[NQB + j][:],
                                     start=(j == 0), stop=(j == NKVB - 1))
                rs_kv = norm_chain(sq_kv, KVLR, "kv")
                # overlap the kv rsqrt chain with the first q-front block
                front_block(0)
                bps = ps1pool.tile([P, SLC], F32, tag="bps", name="bpskv")
                nc.tensor.matmul(bps[:], lhsT=ones_row[:], rhs=rs_kv[:],
                                 start=True, stop=True)
                rb_kv = npool.tile([P, SLC], BF, tag="rbkv", name="rbkv")
                nc.scalar.copy(rb_kv[:], bps[:])
                for j in range(NKVB):
                    nc.vector.tensor_mul(stg_kv[:, j, :], raws[NQB + j][:], rb_kv[:])
                kraw = raws[N_FB - 1]
                ksw = npool.tile([ROPE, SLC], BF, tag="ksw")
                nc.sync.dma_start(out=ksw[0:32, :], in_=kraw[32:64, :])
                nc.sync.dma_start(out=ksw[32:64, :], in_=kraw[0:32, :])
                cql_t = npool.tile([ROPE, SLC], BF, tag="cql")
                sql_t = npool.tile([ROPE, SLC], BF, tag="sql")
                nc.sync.dma_start(out=cql_t[:], in_=cql[:, :])
                nc.sync.dma_start(out=sql_t[:], in_=sql[:, :])
                ka = npool.tile([ROPE, SLC], BF, tag="ka")
                nc.vector.tensor_mul(ka[:], kraw[:ROPE, :], cql_t[:])
                kb_ = npool.tile([ROPE, SLC], BF, tag="kb")
                nc.vector.tensor_mul(kb_[:], ksw[:], sql_t[:])
                nc.vector.tensor_add(stg_kv[0:ROPE, NKVB, :], ka[:], kb_[:])
                nc.sync.dma_start(out=cc_kv[:, :], in_=stg_kv[:, :, :])
                nc.gpsimd.collective_compute(
                    "AllGather", mybir.AluOpType.bypass,
                    replica_groups=[list(range(NCORES))],
                    ins=[cc_kv.opt()], outs=[cc_kv_out.opt()])

                # remaining q-front blocks, then batched q sumsq + norm + gather
                for fb in range(1, NQB):
                    front_block(fb)
                for j in range(NQB):
                    nc.tensor.matmul(sq_q[:], lhsT=ones_bf[:], rhs=sqts[j][:],
                                     start=(j == 0), stop=(j == NQB - 1))
                rs_q = norm_chain(sq_q, QLR, "q")
                bps = ps1pool.tile([P, SLC], F32, tag="bps", name="bpsq")
                nc.tensor.matmul(bps[:], lhsT=ones_row[:], rhs=rs_q[:], start=True, stop=True)
                rb_q = npool.tile([P, SLC], BF, tag="rbq", name="rbq")
                nc.scalar.copy(rb_q[:], bps[:])
                for j in range(NQB):
                    nc.vector.tensor_mul(stg_q[:, j, :], raws[j][:], rb_q[:])
                nc.sync.dma_start(out=cc_q[:, :], in_=stg_q[:, :, :])
                nc.gpsimd.collective_compute(
                    "AllGather", mybir.AluOpType.bypass,
                    replica_groups=[list(range(NCORES))],
                    ins=[cc_q.opt()], outs=[cc_q_out.opt()])

            # ------------- Phase 2a: gathered latent -> q/k/v per-head tensors
            att_pool = tc.tile_pool(name="att_res", bufs=1)
            att = att_pool.__enter__()
            QN = [att.tile([NOPE, S], BF, tag=f"qn{h}", name=f"qn{h}") for h in range(NHC)]
            QR = [att.tile([P, S], BF, tag=f"qr{hp}", name=f"qr{hp}") for hp in range(NHC // 2)]
            KN = [att.tile([NOPE, S], BF, tag=f"kn{h}", name=f"kn{h}") for h in range(NHC)]
            V = [att.tile([P, NHC * VD], BF, tag=f"v{sb}", name=f"v{sb}") for sb in range(NKB)]
            KPE = att.tile([P, S], BF, tag="kpe")  # k_pe duplicated in both halves

            qkv_pool = tc.tile_pool(name="qkv", bufs=1)
            qkv = qkv_pool.__enter__()
            qkv_kv = qkv.tile([P, NKVF, S], BF, tag="qkvkv")
            for r in range(NCORES):
                nc.sync.dma_start(
                    out=qkv_kv[:, :, r * SLC:(r + 1) * SLC],
                    in_=cc_kv_out[r].rearrange("p (j c) -> p j c", j=NKVF))
            nc.vector.tensor_copy(KPE[0:ROPE, :], qkv_kv[0:ROPE, NKVB, :])
            nc.sync.dma_start(out=KPE[ROPE:P, :], in_=qkv_kv[0:ROPE, NKVB, :])

            with (
                tc.tile_pool(name="wb", bufs=1) as wbpool,
                tc.tile_pool(name="rope", bufs=2) as ropepool,
                tc.tile_pool(name="ps2", bufs=4, space="PSUM") as ps2pool,
            ):
                wkvb_t = wbpool.tile([P, NKVB, NHC * (NOPE + VD)], BF, tag="wkvb")
                nc.sync.dma_start(out=wkvb_t[:], in_=Wkvb3[:, :, :])
                wqb_t = wbpool.tile([P, NQB, NHC * QHD], BF, tag="wqb")
                nc.sync.dma_start(out=wqb_t[:], in_=Wqb3[:, :, :])

                # --- kv consumers first (overlap the q AllGather)
                for sc in range(NSC):
                    qsl = slice(sc * SC, (sc + 1) * SC)
                    for h in range(NHC):
                        ps = ps2pool.tile([P, SC], F32, tag="p2", name=f"p2k{h}_{sc}")
                        for j in range(NKVB):
                            nc.tensor.matmul(ps[:], lhsT=wkvb_t[:, j, h * NOPE:(h + 1) * NOPE],
                                             rhs=qkv_kv[:, j, qsl],
                                             start=(j == 0), stop=(j == NKVB - 1))
                        nc.scalar.copy(KN[h][:, qsl], ps[:])
                    # v: all 4 heads per matmul ([seq, 4*vd] layout)
                    voff = NHC * NOPE
                    for sb in range(SC // P):
                        csl2 = slice(sc * SC + sb * P, sc * SC + (sb + 1) * P)
                        psv = ps2pool.tile([P, NHC * VD], F32, tag="pv", name=f"pv{sc}_{sb}")
                        for j in range(NKVB):
                            nc.tensor.matmul(
                                psv[:], lhsT=qkv_kv[:, j, csl2],
                                rhs=wkvb_t[:, j, voff:voff + NHC * VD],
                                start=(j == 0), stop=(j == NKVB - 1))
                        nc.vector.tensor_copy(V[sc * (SC // P) + sb][:], psv[:])

                # --- q consumers
                qkv_q = qkv.tile([P, NQB, S], BF, tag="qkvq")
                for r in range(NCORES):
                    nc.sync.dma_start(
                        out=qkv_q[:, :, r * SLC:(r + 1) * SLC],
                        in_=cc_q_out[r].rearrange("p (j c) -> p j c", j=NQB))
                for sc in range(NSC):
                    qsl = slice(sc * SC, (sc + 1) * SC)
                    for h in range(NHC):
                        ps = ps2pool.tile([P, SC], F32, tag="p2", name=f"p2q{h}_{sc}")
                        for j in range(NQB):
                            nc.tensor.matmul(ps[:], lhsT=wqb_t[:, j, h * P:(h + 1) * P],
                                             rhs=qkv_q[:, j, qsl],
                                             start=(j == 0), stop=(j == NQB - 1))
                        nc.scalar.copy(QN[h][:, qsl], ps[:])
                    # q_pe pairs (2 heads per 128-row tile), rope applied
                    for hp in range(NHC // 2):
                        moff = NHC * P + hp * P
                        ps = ps2pool.tile([P, SC], F32, tag="p2", name=f"p2r{hp}_{sc}")
                        for j in range(NQB):
                            nc.tensor.matmul(ps[:], lhsT=wqb_t[:, j, moff:moff + P],
                                             rhs=qkv_q[:, j, qsl],
                                             start=(j == 0), stop=(j == NQB - 1))
                        qraw = ropepool.tile([P, SC], BF, tag="qraw", name=f"qraw{hp}_{sc}")
                        nc.scalar.copy(qraw[:], ps[:])
                        qsw = ropepool.tile([P, SC], BF, tag="qsw", name=f"qsw{hp}_{sc}")
                        for b0 in range(0, P, ROPE):
                            nc.sync.dma_start(out=qsw[b0:b0 + 32, :], in_=qraw[b0 + 32:b0 + 64, :])
                            nc.sync.dma_start(out=qsw[b0 + 32:b0 + 64, :], in_=qraw[b0:b0 + 32, :])
                        qa_ = ropepool.tile([P, SC], BF, tag="qa_", name=f"qa_{hp}_{sc}")
                        nc.vector.tensor_mul(qa_[:], qraw[:], cqp_t[:, qsl])
                        qb_ = ropepool.tile([P, SC], BF, tag="qb_", name=f"qb_{hp}_{sc}")
                        nc.vector.tensor_mul(qb_[:], qsw[:], sqp_t[:, qsl])
                        nc.vector.tensor_add(QR[hp][:, qsl], qa_[:], qb_[:])
            qkv_pool.__exit__(None, None, None)

            # ------------- Phase 2b+3: attention with Wo interleaved one qc behind
            OSB = [[att.tile([VD, SC], BF, tag=f"osb{h}_{qc}", name=f"osb{h}_{qc}")
                    for h in range(NHC)] for qc in range(NSC)]
            wopool = tc.tile_pool(name="wo", bufs=1)
            wop = wopool.__enter__()
            wo_t = wop.tile([P, NKVB, H], BF, tag="wo")
            nc.sync.dma_start(out=wo_t[:], in_=Wo3[:, :, :])

            with (
                tc.tile_pool(name="attw", bufs=3) as attpool,
                tc.tile_pool(name="oo", bufs=4) as oopool,
                tc.tile_pool(name="dens", bufs=1) as denpool,
                tc.tile_pool(name="ps_o", bufs=1, space="PSUM") as psopool,
                tc.tile_pool(name="ps_l", bufs=4, space="PSUM") as pslpool,
            ):
                def emit_wo(sc):
                    ssl = slice(sc * SC, (sc + 1) * SC)
                    for ho in range(H // P):
                        ps = pslpool.tile([P, SC], F32, tag="pl", name=f"po{sc}_{ho}")
                        for j in range(NKVB):
                            nc.tensor.matmul(ps[:], lhsT=wo_t[:, j, ho * P:(ho + 1) * P],
                                             rhs=OSB[sc][j][:],
                                             start=(j == 0), stop=(j == NKVB - 1))
                        ot = oopool.tile([P, SC], BF, tag="ot", name=f"ot{sc}_{ho}")
                        nc.vector.tensor_copy(ot[:], ps[:])
                        nc.sync.dma_start(out=out3[:, ho, ssl], in_=ot[:])

                def finish(pend):
                    # softmax division + output store for a completed chunk
                    pqc, ocps, rcps = pend
                    for h in range(NHC):
                        bps2 = pslpool.tile([VD, SC], F32, tag="pl", name=f"bps2{pqc}_{h}")
                        nc.tensor.matmul(bps2[:], lhsT=ones_row[:],
                                         rhs=rcps[h], start=True, stop=True)
                        rbb = attpool.tile([VD, SC], F32, tag="rbb", name=f"rbb{pqc}_{h}")
                        nc.vector.tensor_copy(rbb[:], bps2[:])
                        nc.vector.tensor_mul(OSB[pqc][h][:], ocps[h][:], rbb[:])

                pend = None
                for qc in range(NSC):
                    qsl = slice(qc * SC, (qc + 1) * SC)
                    kb_hi = (qc * 4 + 4) if causal else NKB
                    ops = [psopool.tile([VD, SC], F32, tag=f"o{h}", name=f"o{h}_{qc}")
                           for h in range(NHC)]
                    dens = [denpool.tile([P, SC], FR, tag=f"d{h}", name=f"d{h}_{qc}")
                            for h in range(NHC)]
                    for kb in range(kb_hi):
                        ksl = slice(kb * P, (kb + 1) * P)
                        # column restriction: diagonal block d keeps cols >= 128d
                        d = kb - 4 * qc if causal else -1
                        c0 = P * d if (causal and d > 0) else 0
                        n = SC - c0
                        csl = slice(qc * SC + c0, (qc + 1) * SC)
                        if not causal:
                            mt = attpool.tile([P, SC], BF, tag="mt", name=f"mt{qc}_{kb}")
                            nc.sync.dma_start(out=mt[:], in_=maskT[ksl, qsl])
                        pls = []
                        for h in range(NHC):
                            pl = pslpool.tile([P, SC], F32, tag="pl", name=f"pl{qc}_{kb}_{h}")
                            nc.tensor.matmul(pl[:, :n], lhsT=KN[h][:, ksl], rhs=QN[h][:, csl],
                                             start=True, stop=False)
                            nc.tensor.matmul(
                                pl[:, :n], lhsT=KPE[(h % 2) * ROPE:(h % 2 + 1) * ROPE, ksl],
                                rhs=QR[h // 2][(h % 2) * ROPE:(h % 2 + 1) * ROPE, csl],
                                start=False, stop=True)
                            pls.append(pl)
                        for h in range(NHC):
                            pl = pls[h]
                            px = attpool.tile([P, SC], BF, tag="px", name=f"px{qc}_{kb}_{h}")
                            if causal:
                                nc.scalar.activation(px[:, :n], pl[:, :n], AF.Exp)
                                if d >= 0:
                                    nc.vector.tensor_mul(px[:, :n], px[:, :n],
                                                         tri[:, :n])
                            else:
                                pe_ = attpool.tile([P, SC], F32, tag="pe", name=f"pe{qc}_{kb}_{h}")
                                nc.vector.tensor_add(pe_[:], pl[:], mt[:])
                                nc.scalar.activation(px[:], pe_[:], AF.Exp)
                            deng = nc.vector if h < 2 else nc.gpsimd
                            if kb == 0:
                                deng.tensor_copy(dens[h][:], px[:])
                            else:
                                deng.tensor_add(dens[h][:, c0:], dens[h][:, c0:],
                                                px[:, :n])
                            nc.tensor.matmul(ops[h][:, c0:], lhsT=V[kb][:, h * VD:(h + 1) * VD],
                                             rhs=px[:, :n],
                                             start=(kb == 0), stop=(kb == kb_hi - 1))
                    # finish the PREVIOUS chunk's softmax division (its
                    # reciprocal resolved during this chunk's kb loop)
                    if pend is not None:
                        finish(pend)
                    # drain O psum fast (ACT) so next qc can reuse the banks
                    ocps = []
                    for h in range(NHC):
                        ocp = attpool.tile([VD, SC], BF, tag=f"ocp{h}", name=f"ocp{qc}_{h}")
                        nc.scalar.copy(ocp[:], ops[h][:])
                        ocps.append(ocp)
                    # batched softmax denominators: gather the 4 per-head column
                    # sums into one [4, SC] tile (DMA partition shifts), one
                    # reciprocal, shift back for the per-head broadcasts
                    dsb4 = attpool.tile([NHC, SC], F32, tag="dsb", name=f"dsb{qc}")
                    for h in range(NHC):
                        dps = pslpool.tile([1, SC], F32, tag="pl", name=f"dps{qc}_{h}")
                        nc.tensor.matmul(dps[:], lhsT=ones_col[:], rhs=dens[h][:],
                                         start=True, stop=True)
                        dpb = attpool.tile([1, SC], F32, tag=f"dpb{h}", name=f"dpb{qc}_{h}")
                        nc.scalar.copy(dpb[:], dps[:])
                        if h == 0:
                            nc.vector.tensor_copy(dsb4[0:1, :], dpb[:])
                        else:
                            nc.sync.dma_start(out=dsb4[h:h + 1, :], in_=dpb[:])
                    rcp4 = attpool.tile([NHC, SC], FR, tag="rcp", name=f"rcp{qc}")
                    with nc.allow_low_precision(reason="f32r for broadcast matmul"):
                        nc.vector.reciprocal(rcp4[:], dsb4[:])
                    rcps = [rcp4[0:1, :]]
                    for h in range(1, NHC):
                        rcph = attpool.tile([1, SC], FR, tag=f"rcph{h}", name=f"rcph{qc}_{h}")
                        nc.sync.dma_start(out=rcph[:], in_=rcp4[h:h + 1, :])
                        rcps.append(rcph)
                    # Wo of the previous chunk fills PE while rcp4 resolves
                    if pend is not None:
                        emit_wo(pend[0])
                    pend = (qc, ocps, rcps)
                finish(pend)
                emit_wo(NSC - 1)
            wopool.__exit__(None, None, None)
            att_pool.__exit__(None, None, None)

    split_multiwaits(nc)
    return nc


def _pack_front(WqaT, WkvaT):
    """[4096, 1536] + [4096, 576] -> [128, 17*32, 128] bf16, M-tile-major.

    Blocks 0-11: q_a rows; 12-15: kv rows; 16: rope rows (64, zero padded).
    """
    FB_W = [P] * NQB + [P] * NKVB + [ROPE]
    Wfull = np.concatenate([WqaT, WkvaT], axis=1)
    out = np.zeros((P, N_FB * N_KI, P), np.float32)
    off = 0
    for fb, w in enumerate(FB_W):
        blk = Wfull[:, off:off + w].reshape(N_KI, P, w).transpose(1, 0, 2)
        out[:, fb * N_KI:(fb + 1) * N_KI, :w] = blk
        off += w
    return np.ascontiguousarray(out.reshape(P, -1)).astype(NPBF)


def _pack_k(WT, nhw):
    """[K, nhw] -> [128, (K//128)*nhw] bf16: k-tile-major packing."""
    K = WT.shape[0]
    t = WT.reshape(K // P, P, nhw).transpose(1, 0, 2).reshape(P, (K // P) * nhw)
    return np.ascontiguousarray(t).astype(NPBF)


def _rope_tables():
    inv = 1.0 / (BASE ** (np.arange(0, ROPE, 2, dtype=np.float64) / ROPE))
    t = np.arange(S, dtype=np.float64)
    fr_ = np.outer(t, inv)
    emb = np.concatenate([fr_, fr_], axis=1)
    cos = np.cos(emb).T.astype(np.float32)     # [64, S]
    sin = np.sin(emb).T.astype(np.float32)
    ssin = sin.copy()
    ssin[:32] *= -1.0
    cqp = np.concatenate([cos, cos], axis=0)   # [128, S] (head pair stacked)
    sqp = np.concatenate([ssin, ssin], axis=0)
    return cqp.astype(NPBF), sqp.astype(NPBF)


def kernel(hidden_states, attention_mask, Wqa, qa_ln_w, Wqb, Wkva, kva_ln_w, Wkvb, Wo):
    hidden_states = np.asarray(hidden_states, np.float32)
    attention_mask = np.asarray(attention_mask, np.float32)
    Wqa = np.asarray(Wqa, np.float32)
    Wqb = np.asarray(Wqb, np.float32)
    Wkva = np.asarray(Wkva, np.float32)
    Wkvb = np.asarray(Wkvb, np.float32)
    Wo = np.asarray(Wo, np.float32)
    qa_ln_w = np.asarray(qa_ln_w, np.float32)
    kva_ln_w = np.asarray(kva_ln_w, np.float32)

    mask = attention_mask[0, 0]
    tril = np.tril(np.ones((S, S), bool))
    causal = bool(np.array_equal(mask, np.where(tril, 0.0, -1e9).astype(np.float32)))

    hT = np.ascontiguousarray(hidden_states[0].T)          # [H, S] f32
    Wp = _pack_front(np.ascontiguousarray(Wqa.T), np.ascontiguousarray(Wkva.T))
    cqp, sqp = _rope_tables()

    Wqb_eff = (Wqb * qa_ln_w[None, :]).astype(np.float32) * np.float32(SCALE)
    Wkvb_eff = (Wkvb * kva_ln_w[None, :]).astype(np.float32)

    in_maps = []
    for c in range(NCORES):
        hsl = slice(c * NHC * QHD, (c + 1) * NHC * QHD)
        ksl = slice(c * NHC * (NOPE + VD), (c + 1) * NHC * (NOPE + VD))
        osl = slice(c * NHC * VD, (c + 1) * NHC * VD)
        # reorder Wqb slice columns: [h0n h1n h2n h3n | h0r h1r | h2r h3r]
        Wq_sl = np.ascontiguousarray(Wqb_eff[hsl].T)       # [1536, 768]
        perm = (
            [h * QHD + i for h in range(NHC) for i in range(NOPE)]
            + [h * QHD + NOPE + i for h in range(NHC) for i in range(ROPE)]
        )
        Wq_perm = np.ascontiguousarray(Wq_sl[:, perm])
        # reorder Wkvb slice columns: [h0n h1n h2n h3n | h0v h1v h2v h3v]
        Wkv_sl = np.ascontiguousarray(Wkvb_eff[ksl].T)     # [512, 1024]
        kperm = (
            [h * (NOPE + VD) + i for h in range(NHC) for i in range(NOPE)]
            + [h * (NOPE + VD) + NOPE + i for h in range(NHC) for i in range(VD)]
        )
        Wkv_perm = np.ascontiguousarray(Wkv_sl[:, kperm])
        csl = slice(c * SLC, (c + 1) * SLC)
        im = {
            "hTs": np.ascontiguousarray(hT[:, csl]).astype(NPBF),
            "Wp": Wp,
            "Wqb_p": _pack_k(Wq_perm, NHC * QHD),
            "Wkvb_p": _pack_k(Wkv_perm, NHC * (NOPE + VD)),
            "Wo_p": _pack_k(np.ascontiguousarray(Wo[:, osl].T), H),
            "cqp": cqp,
            "sqp": sqp,
            "cql": np.ascontiguousarray(cqp[:ROPE, csl]),
            "sql": np.ascontiguousarray(sqp[:ROPE, csl]),
        }
        in_maps.append(im)
    if not causal:
        maskT = np.ascontiguousarray(mask.T).astype(NPBF)
        for c in range(NCORES):
            in_maps[c]["maskT"] = maskT

    nc = build(causal)
    trace = bool(os.environ.get("KPROF"))
    res = run_bass_kernel_spmd(nc, in_maps, list(range(NCORES)), trace=trace)
    if trace:
        print(f"HW exec time: {res.exec_time_ns} ns (mean {res.mean_exec_time_ns}, "
              f"max core {res.max_exec_time_core_id})")
        print(f"profile_json: {res.profile_json}")
        if res.instructions_and_trace:
            print(f"trace_path: {res.instructions_and_trace[1]}")
    acc = np.zeros((P, (H // P) * S), np.float32)
    for c in range(NCORES):
        acc += np.asarray(res.results[c]["outT"], dtype=np.float32)
    # outT is [128, 32, 2048] p-major: row ho*128+p = acc[p, ho, :]
    full = acc.reshape(P, H // P, S).transpose(1, 0, 2).reshape(H, S)
    return np.ascontiguousarray(full.T)[None, :, :].astype(np.float32)
